# revision 4
# baseline (speedup 1.0000x reference)
"""Trainium2 Bass kernel for nn_Causal_model_vae (moe_routing).

Model (per row of data [N, 128]):
    mu_h     = LeakyReLU(data @ mu_W1 + mu_b1, 0.01) @ mu_W2 + mu_b2        [N, 64]
    logvar_h = same with lv_* weights                                       [N, 64]
    h        = eps * exp(0.5 * logvar_h) + mu_h                             [N, 64]
    reconst  = LeakyReLU(h @ dec_W1[s] + dec_b1[s], 0.01) @ dec_W2[s] + dec_b2[s]
returns (reconst [N,128], mu_h, logvar_h, h)

Strategy:
  - Pure data-parallel over 8 NeuronCores: rows sharded N/8 per core.
  - Per shard, rows are SORTED BY GROUP on the host and each group's segment
    padded to a multiple of 512 (padded counts shared across cores so one
    SPMD program serves all 8). Each 512-row chunk then belongs to exactly
    one group, so the decoder runs ONE small MLP per chunk - no masks, no
    wasted compute, and group boundaries are baked into the program at build
    time (we compile after seeing `s`).
  - Everything lives transposed on device: feature dim on SBUF partitions,
    rows on the free axis, so matmuls contract along partitions with the
    small weights stationary. Host pre/post-transposes (cheap marshaling).
  - Matmuls run in float32r (TF32-like, ~11 mantissa bits, 4x faster than
    fp32 on the PE). Inputs are pre-rounded host-side; on-chip activation
    outputs that feed matmuls are written as f32r directly.
"""

import numpy as np

N = 524288
DIM_X = 128
DIM_H = 64
NUM_S = 8
N_CORES = 8
SHARD = N // N_CORES          # 65536 rows per core
CHUNK = 512                   # rows per matmul (PSUM bank = 512 fp32)
MACRO = 2048                  # rows per DMA macro-tile

_prog_cache = {}


def _round_f32r(x: np.ndarray) -> np.ndarray:
    """Round fp32 to f32r precision (round-to-nearest-even, keep 11 explicit
    mantissa bits) so DMA-fed f32r matmul operands match engine-rounded ones."""
    u = np.ascontiguousarray(x, np.float32).view(np.uint32)
    lsb = (u >> 12) & np.uint32(1)
    r = (u + np.uint32(0x7FF) + lsb) & np.uint32(0xFFFFF000)
    return r.view(np.float32)


def _build_program(chunk_groups: tuple, r_pad: int):
    import concourse.mybir as mybir
    import concourse.tile as tile
    from concourse import bacc

    f32 = mybir.dt.float32
    f32r = mybir.dt.float32r
    LRELU = mybir.ActivationFunctionType.Lrelu
    IDENT = mybir.ActivationFunctionType.Identity
    EXP = mybir.ActivationFunctionType.Exp

    nc = bacc.Bacc(None, debug=False)

    dataT = nc.dram_tensor("dataT", [DIM_X, r_pad], f32r, kind="ExternalInput")
    epsT = nc.dram_tensor("epsT", [DIM_H, r_pad], f32, kind="ExternalInput")
    enc1_w = nc.dram_tensor("enc1_w", [DIM_X, 128], f32r, kind="ExternalInput")
    enc1_b = nc.dram_tensor("enc1_b", [128, 1], f32, kind="ExternalInput")
    enc2_w = nc.dram_tensor("enc2_w", [128, 128], f32r, kind="ExternalInput")
    enc2_b = nc.dram_tensor("enc2_b", [128, 1], f32, kind="ExternalInput")
    dec1_w = nc.dram_tensor("dec1_w", [DIM_H, NUM_S, DIM_H], f32r, kind="ExternalInput")
    dec1_b = nc.dram_tensor("dec1_b", [DIM_H, NUM_S], f32, kind="ExternalInput")
    dec2_w = nc.dram_tensor("dec2_w", [DIM_H, NUM_S, DIM_X], f32r, kind="ExternalInput")
    dec2_b = nc.dram_tensor("dec2_b", [DIM_X, NUM_S], f32, kind="ExternalInput")

    reconT = nc.dram_tensor("reconT", [DIM_X, r_pad], f32, kind="ExternalOutput")
    muT = nc.dram_tensor("muT", [DIM_H, r_pad], f32, kind="ExternalOutput")
    lvT = nc.dram_tensor("lvT", [DIM_H, r_pad], f32, kind="ExternalOutput")
    hT = nc.dram_tensor("hT", [DIM_H, r_pad], f32, kind="ExternalOutput")

    n_chunks = r_pad // CHUNK
    assert len(chunk_groups) == n_chunks

    with tile.TileContext(nc) as tc:
        with (
            tc.tile_pool(name="wpool", bufs=1) as wpool,
            tc.tile_pool(name="inp", bufs=3) as inp,
            tc.tile_pool(name="mid", bufs=3) as mid,
            tc.tile_pool(name="outp", bufs=3) as outp,
            tc.tile_pool(name="ps", bufs=2, space="PSUM") as ps,
        ):
            w1 = wpool.tile([DIM_X, 128], f32r)
            b1 = wpool.tile([128, 1], f32)
            w2 = wpool.tile([128, 128], f32r)
            b2 = wpool.tile([128, 1], f32)
            dw1 = wpool.tile([DIM_H, NUM_S, DIM_H], f32r)
            db1 = wpool.tile([DIM_H, NUM_S], f32)
            dw2 = wpool.tile([DIM_H, NUM_S, DIM_X], f32r)
            db2 = wpool.tile([DIM_X, NUM_S], f32)
            nc.sync.dma_start(w1[:], enc1_w[:])
            nc.sync.dma_start(b1[:], enc1_b[:])
            nc.sync.dma_start(w2[:], enc2_w[:])
            nc.sync.dma_start(b2[:], enc2_b[:])
            nc.sync.dma_start(dw1[:], dec1_w[:])
            nc.sync.dma_start(db1[:], dec1_b[:])
            nc.sync.dma_start(dw2[:], dec2_w[:])
            nc.sync.dma_start(db2[:], dec2_b[:])

            for m0 in range(0, r_pad, MACRO):
                mw = min(MACRO, r_pad - m0)
                x_mac = inp.tile([DIM_X, MACRO], f32r, tag="x_mac")
                e_mac = inp.tile([DIM_H, MACRO], f32, tag="e_mac")
                nc.sync.dma_start(x_mac[:, :mw], dataT[:, m0:m0 + mw])
                nc.sync.dma_start(e_mac[:, :mw], epsT[:, m0:m0 + mw])

                mulv_mac = outp.tile([128, MACRO], f32, tag="mulv_mac")
                h_mac = outp.tile([DIM_H, MACRO], f32r, tag="h_mac")
                rec_mac = outp.tile([DIM_X, MACRO], f32, tag="rec_mac")

                for c0 in range(0, mw, CHUNK):
                    g = chunk_groups[(m0 + c0) // CHUNK]
                    sl = slice(c0, c0 + CHUNK)

                    # encoder layer 1: [mu_pre; lv_pre]^T = enc1_w.T @ x
                    p_e1 = ps.tile([128, CHUNK], f32, tag="p_e1")
                    nc.tensor.matmul(p_e1[:], w1[:], x_mac[:, sl],
                                     start=True, stop=True)
                    a1 = mid.tile([128, CHUNK], f32r, tag="a1")
                    nc.scalar.activation(a1[:], p_e1[:], LRELU,
                                         bias=b1[:], scale=1.0, alpha=0.01)

                    # encoder layer 2 (block-diag): [mu_h; logvar]^T
                    p_e2 = ps.tile([128, CHUNK], f32, tag="p_e2")
                    nc.tensor.matmul(p_e2[:], w2[:], a1[:],
                                     start=True, stop=True)
                    nc.scalar.activation(mulv_mac[:, sl], p_e2[:], IDENT,
                                         bias=b2[:], scale=1.0)

                    # std = exp(0.5 * logvar); h = eps*std + mu
                    std = mid.tile([DIM_H, CHUNK], f32, tag="std")
                    nc.scalar.activation(std[:], mulv_mac[64:128, sl], EXP,
                                         bias=0.0, scale=0.5)
                    nc.vector.tensor_mul(std[:], std[:], e_mac[:, sl])
                    nc.vector.tensor_add(h_mac[:, sl], std[:],
                                         mulv_mac[0:64, sl])

                    # decoder (single group g for this chunk)
                    p_d1 = ps.tile([DIM_H, CHUNK], f32, tag="p_d1")
                    nc.tensor.matmul(p_d1[:], dw1[:, g, :], h_mac[:, sl],
                                     start=True, stop=True)
                    d1 = mid.tile([DIM_H, CHUNK], f32r, tag="d1")
                    nc.scalar.activation(d1[:], p_d1[:], LRELU,
                                         bias=db1[:, g:g + 1], scale=1.0,
                                         alpha=0.01)
                    p_d2 = ps.tile([DIM_X, CHUNK], f32, tag="p_d2")
                    nc.tensor.matmul(p_d2[:], dw2[:, g, :], d1[:],
                                     start=True, stop=True)
                    nc.scalar.activation(rec_mac[:, sl], p_d2[:], IDENT,
                                         bias=db2[:, g:g + 1], scale=1.0)

                nc.sync.dma_start(muT[:, m0:m0 + mw], mulv_mac[0:64, :mw])
                nc.sync.dma_start(lvT[:, m0:m0 + mw], mulv_mac[64:128, :mw])
                nc.sync.dma_start(hT[:, m0:m0 + mw],
                                  h_mac[:, :mw].bitcast(f32))
                nc.sync.dma_start(reconT[:, m0:m0 + mw], rec_mac[:, :mw])

    nc.compile()
    return nc


def _make_runner(nc):
    """Build a sharded jit over the compiled Bass program (axon/PJRT path).

    Mirrors concourse.bass2jax.run_bass_via_pjrt but keeps the jit + metadata
    so callers can re-execute with device-resident inputs (for timing).
    """
    import jax
    import numpy as jnp_np  # noqa: F401
    from jax.sharding import Mesh, PartitionSpec, NamedSharding
    from jax.experimental.shard_map import shard_map
    import concourse.mybir as mybir
    from concourse.bass2jax import (
        install_neuronx_cc_hook, _bass_exec_p, partition_id_tensor,
    )

    install_neuronx_cc_hook()
    partition_name = (nc.partition_id_tensor.name
                      if nc.partition_id_tensor else None)

    in_names, out_names, out_avals, zero_outs = [], [], [], []
    for alloc in nc.m.functions[0].allocations:
        if not isinstance(alloc, mybir.MemoryLocationSet):
            continue
        name = alloc.memorylocations[0].name
        if alloc.kind == "ExternalInput":
            if name != partition_name:
                in_names.append(name)
        elif alloc.kind == "ExternalOutput":
            shape = tuple(alloc.tensor_shape)
            dtype = mybir.dt.np(alloc.dtype)
            out_names.append(name)
            out_avals.append(jax.core.ShapedArray(shape, dtype))
            zero_outs.append((shape, dtype))
    n_params = len(in_names)
    n_outs = len(out_names)
    in_names = in_names + out_names
    if partition_name is not None:
        in_names.append(partition_name)
    donate = tuple(range(n_params, n_params + n_outs))

    def _body(*args):
        operands = list(args)
        if partition_name is not None:
            operands.append(partition_id_tensor())
        outs = _bass_exec_p.bind(
            *operands,
            out_avals=tuple(out_avals),
            in_names=tuple(in_names),
            out_names=tuple(out_names),
            lowering_input_output_aliases=(),
            sim_require_finite=True,
            sim_require_nnan=True,
            nc=nc,
        )
        return tuple(outs)

    devices = jax.devices()[:N_CORES]
    mesh = Mesh(np.asarray(devices), ("core",))
    sharded = jax.jit(
        shard_map(
            _body, mesh=mesh,
            in_specs=(PartitionSpec("core"),) * (n_params + n_outs),
            out_specs=(PartitionSpec("core"),) * n_outs,
            check_rep=False,
        ),
        donate_argnums=donate,
        keep_unused=True,
    )
    sharding = NamedSharding(mesh, PartitionSpec("core"))
    return {
        "jit": sharded, "in_names": in_names[:n_params],
        "out_names": out_names, "zero_outs": zero_outs,
        "sharding": sharding,
    }


def _run(runner, in_maps, bench_iters=0):
    """Execute; returns (per-core results dict list, per-iter exec ns or None)."""
    import jax
    import time

    jit = runner["jit"]
    concat_in = [
        np.concatenate([m[name] for m in in_maps], axis=0)
        for name in runner["in_names"]
    ]
    concat_zeros = [
        np.zeros((N_CORES * sh[0], *sh[1:]), dt)
        for sh, dt in runner["zero_outs"]
    ]
    bench_ns = None
    if bench_iters:
        dev_in = [jax.device_put(a, runner["sharding"]) for a in concat_in]
        outs = jit(*dev_in, *concat_zeros)
        jax.block_until_ready(outs)
        # chain donated outputs; inputs stay device-resident
        t0 = time.perf_counter()
        for _ in range(bench_iters):
            outs = jit(*dev_in, *outs)
        jax.block_until_ready(outs)
        bench_ns = (time.perf_counter() - t0) / bench_iters * 1e9
        # one more clean pass for the returned results (outs hold garbage of
        # repeated application? no - kernel is pure f(inputs)->outputs, outs
        # are identical every iteration)
        out_arrs = outs
    else:
        out_arrs = jit(*concat_in, *concat_zeros)

    results = []
    for c in range(N_CORES):
        d = {}
        for i, name in enumerate(runner["out_names"]):
            sh, dt = runner["zero_outs"][i]
            d[name] = np.asarray(out_arrs[i]).reshape(N_CORES, *sh)[c]
        results.append(d)
    return results, bench_ns


def kernel(**inputs):
    bench_iters = int(inputs.pop("_bench", 0))

    data = np.ascontiguousarray(np.asarray(inputs["data"]), dtype=np.float32)
    s = np.asarray(inputs["s"]).astype(np.int64).ravel()
    eps = np.ascontiguousarray(np.asarray(inputs["eps"]), dtype=np.float32)
    f32 = np.float32

    def a(name):
        return np.asarray(inputs[name], dtype=np.float32)

    mu_W1, mu_b1, mu_W2, mu_b2 = a("mu_W1"), a("mu_b1"), a("mu_W2"), a("mu_b2")
    lv_W1, lv_b1, lv_W2, lv_b2 = a("lv_W1"), a("lv_b1"), a("lv_W2"), a("lv_b2")
    dec_W1, dec_b1 = a("dec_W1"), a("dec_b1")
    dec_W2, dec_b2 = a("dec_W2"), a("dec_b2")

    # ---- host-side routing: per-shard stable sort by group + padding ----
    s_sh = s.reshape(N_CORES, SHARD)
    counts = np.stack([np.bincount(s_sh[c], minlength=NUM_S)
                       for c in range(N_CORES)])            # [8 cores, 8 groups]
    pad_counts = (-(-counts.max(axis=0) // CHUNK)) * CHUNK  # per group, shared
    offs = np.concatenate([[0], np.cumsum(pad_counts)])
    r_pad = int(offs[-1])
    chunk_groups = tuple(
        g for g in range(NUM_S) for _ in range(pad_counts[g] // CHUNK)
    )

    perms, poss = [], []
    for c in range(N_CORES):
        perm = np.argsort(s_sh[c], kind="stable")
        pos = np.concatenate(
            [offs[g] + np.arange(counts[c, g]) for g in range(NUM_S)]
        ).astype(np.int64)
        perms.append(perm)
        poss.append(pos)

    # ---- shared (replicated) weight blobs ----
    enc1_w = _round_f32r(np.concatenate([mu_W1, lv_W1], axis=1))   # [128,128]
    enc1_b = np.concatenate([mu_b1, lv_b1]).reshape(128, 1).astype(f32)
    enc2_w = np.zeros((128, 128), f32)
    enc2_w[0:64, 0:64] = mu_W2
    enc2_w[64:128, 64:128] = lv_W2
    enc2_w = _round_f32r(enc2_w)
    enc2_b = np.concatenate([mu_b2, lv_b2]).reshape(128, 1).astype(f32)
    dec1_w = _round_f32r(np.ascontiguousarray(dec_W1.transpose(1, 0, 2)))
    dec1_b = np.ascontiguousarray(dec_b1.T).astype(f32)            # [64, 8]
    dec2_w = _round_f32r(np.ascontiguousarray(dec_W2.transpose(1, 0, 2)))
    dec2_b = np.ascontiguousarray(dec_b2.T).astype(f32)            # [128, 8]

    weights = {
        "enc1_w": enc1_w, "enc1_b": enc1_b,
        "enc2_w": enc2_w, "enc2_b": enc2_b,
        "dec1_w": dec1_w, "dec1_b": dec1_b,
        "dec2_w": dec2_w, "dec2_b": dec2_b,
    }

    # ---- per-core gathered, transposed, padded inputs ----
    in_maps = []
    for c in range(N_CORES):
        lo = c * SHARD
        tmp = np.zeros((r_pad, DIM_X), f32)
        tmp[poss[c]] = data[lo:lo + SHARD][perms[c]]
        dataT = _round_f32r(np.ascontiguousarray(tmp.T))
        tmp = np.zeros((r_pad, DIM_H), f32)
        tmp[poss[c]] = eps[lo:lo + SHARD][perms[c]]
        epsT = np.ascontiguousarray(tmp.T)
        in_maps.append({"dataT": dataT, "epsT": epsT, **weights})

    # ---- build (or reuse) + run the SPMD program ----
    key = (chunk_groups, r_pad)
    if key not in _prog_cache:
        nc = _build_program(chunk_groups, r_pad)
        _prog_cache[key] = _make_runner(nc)
    runner = _prog_cache[key]

    results, bench_ns = _run(runner, in_maps, bench_iters=bench_iters)
    kernel._bench_ns = bench_ns

    # ---- unshard / unsort / untranspose ----
    reconst = np.empty((N, DIM_X), f32)
    mu_h = np.empty((N, DIM_H), f32)
    logvar = np.empty((N, DIM_H), f32)
    h_samp = np.empty((N, DIM_H), f32)
    for c in range(N_CORES):
        r = results[c]
        lo = c * SHARD
        rows = perms[c] + lo
        reconst[rows] = r["reconT"].T[poss[c]]
        mu_h[rows] = r["muT"].T[poss[c]]
        logvar[rows] = r["lvT"].T[poss[c]]
        h_samp[rows] = r["hT"].T[poss[c]]

    return (reconst, mu_h, logvar, h_samp)
